# revision 34
# baseline (speedup 1.0000x reference)
"""BrainGNN forward pass on 8 Trainium2 NeuronCores, data-parallel over batch.

Algorithm notes (validated against the jax reference on CPU):
  - Top-k pooling keeps the 400-node layout and masks dropped nodes instead of
    gathering: the final readouts (max/mean) are invariant to node order, so
    only the kept SET matters.  keep = (rank < K) with
    rank_j = #{i: s_i > s_j} = 399 - #{i: s_j > s_i}, computed from a
    comparison matrix + ones-matmul column sums.
  - NNConv's per-node weight W[n] = (relu(pos @ Wa) @ Wb).reshape(...) has the
    identity as pos, so W[n] = sum_c relu(Wa)[n,c] * B[c]: rank-8 across nodes.
    ht = per-node h @ W[n] becomes 8 dense matmuls G_c = h @ B_c plus a small
    per-partition linear combination.
  - augment_adj (A@A) values are never materialized: msg2 = w2 @ ht2 is two
    fp32 sandwich matmuls t = al @ ht2 (keep-masked) and msg2 = t^T-layout @
    alT, plus a diagonal correction (w2's natural diagonal is replaced by
    weight-1 self-loops in the reference).  The A@A STRUCTURE (neighbor
    counts) is one exact bf16 indicator matmul chain.
  - BatchNorm in the head needs full-batch stats: per-core readouts are
    AllGathered and every core computes the identical tiny head.

Precision notes (this hardware):
  - fp32r matmuls are tf32-class (~1e-3 rel) — unusable: the output gate
    needs ~1e-6 value accuracy (BatchNorm + near-zero logits), and pool2
    score gaps are ~1e-6.  EVERYTHING runs in true fp32 matmuls (exact to
    ~1e-7), which this compiler keeps as fp32 for any N.
  - fp32r PE transposes are lossy; exact transposes are plain fp32 matmuls
    with an identity RHS (single nonzero product per output, no rounding).
    alT (the transposed adjacency) comes from the host.
  - Row<->col layout moves and partition broadcasts go through DRAM DMA
    roundtrips (bit-exact; DRAM-side stride-0 APs are legal, SBUF-side not).
  - Score ranking uses pre-sigmoid scores; row and column layouts of a score
    vector always come from the same DRAM bytes so comparisons are
    self-consistent.  Pool2 rank masking is additive (y*keep + (keep-1)*C),
    never (y+C)*keep-C, which would quantize at ulp(C) >> the 1e-6 gaps.
"""

import math
import numpy as np

NCORES = 8
B = 64
BL = B // NCORES          # graphs per core
R = 400
KC = 8                    # K_COMM rank of the per-node weight factorization
D1 = 32
D2 = 32
D3 = 512
K1 = math.ceil(0.9 * R)   # 360
K2 = math.ceil(0.9 * K1)  # 324
EPS = 1e-5
BIG = 2.0               # masked-max offset; |h| < 0.5 validated on CPU
CBIG = 512.0            # pool2 rank masking offset (exactly representable)

# 400 = 3*128 + 16 partition chunks
CH = [(0, 128), (128, 128), (256, 128), (384, 16)]


def build_nc(n_cores=NCORES):
    import concourse.bass as bass
    import concourse.mybir as mybir
    from concourse import tile

    F32 = mybir.dt.float32
    AX = mybir.AxisListType
    OP = mybir.AluOpType
    AF = mybir.ActivationFunctionType

    nc = bass.Bass()
    # scratch semaphore for _legalize_waits carrier updates; allocated
    # before the TileContext so tile sem assignment can't reuse the slot
    nc._lgl_dummy_sem = nc.alloc_semaphore("lgl_dummy_sem")

    xl = nc.dram_tensor("xl", [BL, R, R], F32, kind="ExternalInput")
    al = nc.dram_tensor("al", [BL, R, R], F32, kind="ExternalInput")
    altd = nc.dram_tensor("altd", [BL, R, R], F32, kind="ExternalInput")
    w1a = nc.dram_tensor("w1a", [R, KC], F32, kind="ExternalInput")
    bc1 = nc.dram_tensor("bc1", [R, D1 * KC], F32, kind="ExternalInput")
    b1d = nc.dram_tensor("b1d", [D1], F32, kind="ExternalInput")
    p1d = nc.dram_tensor("p1d", [D1], F32, kind="ExternalInput")
    w2a = nc.dram_tensor("w2a", [R, KC], F32, kind="ExternalInput")
    bc2 = nc.dram_tensor("bc2", [D1, D2 * KC], F32, kind="ExternalInput")
    b2d = nc.dram_tensor("b2d", [D2], F32, kind="ExternalInput")
    p2d = nc.dram_tensor("p2d", [D2], F32, kind="ExternalInput")
    fc1wd = nc.dram_tensor("fc1wd", [4 * D1, D2], F32, kind="ExternalInput")
    fc1bd = nc.dram_tensor("fc1bd", [D2], F32, kind="ExternalInput")
    g1d = nc.dram_tensor("g1d", [D2], F32, kind="ExternalInput")
    be1d = nc.dram_tensor("be1d", [D2], F32, kind="ExternalInput")
    fc2wd = nc.dram_tensor("fc2wd", [D2, D3], F32, kind="ExternalInput")
    fc2bd = nc.dram_tensor("fc2bd", [D3], F32, kind="ExternalInput")
    g2d = nc.dram_tensor("g2d", [D3], F32, kind="ExternalInput")
    be2d = nc.dram_tensor("be2d", [D3], F32, kind="ExternalInput")
    fc3wd = nc.dram_tensor("fc3wd", [D3, 2], F32, kind="ExternalInput")
    fc3bd = nc.dram_tensor("fc3bd", [2], F32, kind="ExternalInput")
    outd = nc.dram_tensor("out", [B, 2], F32, kind="ExternalOutput")

    from contextlib import ExitStack

    with tile.TileContext(nc) as tc, ExitStack() as es:
        cons = es.enter_context(tc.tile_pool(name="cons", bufs=1))
        work = es.enter_context(tc.tile_pool(name="work", bufs=2))
        dram = es.enter_context(tc.tile_pool(name="dram", bufs=1, space="DRAM"))
        psets = []
        for par in range(2):
            psets.append((
                es.enter_context(tc.tile_pool(name=f"pbig{par}", bufs=1, space="PSUM")),
                es.enter_context(tc.tile_pool(name=f"pg{par}", bufs=1, space="PSUM")),
                es.enter_context(tc.tile_pool(name=f"pacc{par}", bufs=1, space="PSUM")),
                es.enter_context(tc.tile_pool(name=f"prep{par}", bufs=1, space="PSUM")),
            ))
        pbig, pg, pacc, prep = psets[0]

        # ---------------- constants / weights ----------------
        ones128 = cons.tile([128, 128], F32, tag="ones128")
        nc.vector.memset(ones128[:], 1.0)
        ones_r = cons.tile([1, 128], F32, tag="ones_r")
        nc.vector.memset(ones_r[:], 1.0)
        BF16 = mybir.dt.bfloat16
        ones_bf = cons.tile([128, D1], BF16, tag="ones_bf")
        nc.vector.memset(ones_bf[:], 1.0)
        ones_rb = cons.tile([1, D1], BF16, tag="ones_rb")
        nc.vector.memset(ones_rb[:], 1.0)

        a1t, a2t, bc1t = [], [], []
        for c, (o, n) in enumerate(CH):
            t = cons.tile([n, KC], F32, tag=f"a1t{c}")
            nc.sync.dma_start(t[:], w1a[o:o + n, :])
            nc.scalar.activation(t[:], t[:], AF.Relu)
            a1t.append(t)
            t2 = cons.tile([n, KC], F32, tag=f"a2t{c}")
            nc.sync.dma_start(t2[:], w2a[o:o + n, :])
            nc.scalar.activation(t2[:], t2[:], AF.Relu)
            a2t.append(t2)
            tb0 = cons.tile([n, D1 * KC], F32, tag=f"bc1f{c}")
            nc.sync.dma_start(tb0[:], bc1[o:o + n, :])
            bc1t.append(tb0)
        bc2f = cons.tile([D1, D2 * KC], F32, tag="bc2f")
        nc.sync.dma_start(bc2f[:], bc2[:, :])

        def colvec(d, name, nrow):
            t = cons.tile([nrow, 1], F32, tag=name)
            nc.sync.dma_start(t[:], d[:].unsqueeze(1))
            return t

        b1t = colvec(b1d, "b1t", D1)
        p1t = colvec(p1d, "p1t", D1)
        b2t = colvec(b2d, "b2t", D2)
        p2t = colvec(p2d, "p2t", D2)
        fc1bt = colvec(fc1bd, "fc1bt", D2)
        g1t = colvec(g1d, "g1t", D2)
        be1t = colvec(be1d, "be1t", D2)
        fc3bt = colvec(fc3bd, "fc3bt", 2)

        fc1wt = cons.tile([4 * D1, D2], F32, tag="fc1wt")
        nc.sync.dma_start(fc1wt[:], fc1wd[:, :])
        fc2wt = cons.tile([D2, D3], F32, tag="fc2wt")
        nc.sync.dma_start(fc2wt[:], fc2wd[:, :])
        # [512] vectors -> [128, 4] (partition-major chunks)
        fc2b4 = cons.tile([128, 4], F32, tag="fc2b4")
        nc.sync.dma_start(fc2b4[:], fc2bd[:].rearrange("(c p) -> p c", p=128))
        g24 = cons.tile([128, 4], F32, tag="g24")
        nc.sync.dma_start(g24[:], g2d[:].rearrange("(c p) -> p c", p=128))
        be24 = cons.tile([128, 4], F32, tag="be24")
        nc.sync.dma_start(be24[:], be2d[:].rearrange("(c p) -> p c", p=128))
        # fc3w [512,2] -> [128, (4,2)]
        fc3wt = cons.tile([128, 8], F32, tag="fc3wt")
        nc.sync.dma_start(fc3wt[:].rearrange("p (c o) -> p c o", o=2),
                          fc3wd[:, :].rearrange("(c p) o -> p c o", p=128))

        I128 = cons.tile([128, 128], F32, tag="I128")
        nc.gpsimd.affine_select(I128[:], ones128[:], pattern=[[-1, 128]],
                                compare_op=OP.is_equal, fill=0.0,
                                base=0, channel_multiplier=1)
        notI = cons.tile([128, 128], F32, tag="notI")
        nc.gpsimd.affine_select(notI[:], ones128[:], pattern=[[-1, 128]],
                                compare_op=OP.not_equal, fill=0.0,
                                base=0, channel_multiplier=1)

        ztile = cons.tile([128, BL], F32, tag="ztile")
        eps128 = cons.tile([128, 1], F32, tag="eps128")
        nc.vector.memset(eps128[:], EPS)

        # ---------------- per-graph pipeline, software-pipelined ----------
        # Engines execute their queues in emission order, so emitting graphs
        # back-to-back serializes the whole batch on the dependency spine.
        # Split the graph body into stages and emit with a skew so two
        # graphs are in flight; parity PSUM pools + bufs=2 SBUF tags keep
        # their resources disjoint.

        def st_load(b, S):
            S["x32"], S["at"], S["alT"] = [], [], []
            for c, (o, n) in enumerate(CH):
                t0 = work.tile([n, R], F32, tag=f"xf{c}")
                nc.sync.dma_start(t0[:], xl[b, o:o + n, :])
                S["x32"].append(t0)
                t = work.tile([n, R], F32, tag=f"at{c}")
                nc.sync.dma_start(t[:], al[b, o:o + n, :])
                S["at"].append(t)
                t = work.tile([n, R], F32, tag=f"alT{c}")
                nc.gpsimd.dma_start(t[:], altd[b, o:o + n, :])
                S["alT"].append(t)

        def st_cnt_conv1(b, S):
            pbig, pg, pacc, prep = psets[b % 2]
            ind = []
            cntp = pacc.tile([D1, R], F32, tag="pacc")
            for jc, (jo, jn) in enumerate(CH):
                t = work.tile([jn, R], BF16, tag=f"ind{jc}")
                nc.gpsimd.tensor_scalar(t[:], S["alT"][jc][:], 0.0, None, op0=OP.is_gt)
                ind.append(t)
                nc.tensor.matmul(cntp[:], ones_bf[:jn, :D1], t[:],
                                 start=(jc == 0), stop=(jc == 3))
            S["ind"] = ind
            recip1 = work.tile([D1, R], F32, tag="recip1")
            nc.vector.reciprocal(recip1[:], cntp[:])
            S["recip1"] = recip1
            ht1 = []
            for mc, (mo, mn) in enumerate(CH):
                gp = pg.tile([mn, D1 * KC], F32, tag="pG")
                for dc, (do, dn) in enumerate(CH):
                    nc.tensor.matmul(gp[:], S["x32"][dc][:, mo:mo + mn], bc1t[dc][:],
                                     start=(dc == 0), stop=(dc == 3))
                prod = work.tile([mn, D1 * KC], F32, tag="prod")
                abc = a1t[mc][:].unsqueeze(1).broadcast_to((mn, D1, KC))
                nc.vector.tensor_tensor(prod[:].rearrange("p (o c) -> p o c", c=KC),
                                        gp[:].rearrange("p (o c) -> p o c", c=KC),
                                        abc, op=OP.mult)
                t = work.tile([mn, D1], F32, tag=f"ht1_{mc}")
                nc.vector.tensor_reduce(t[:], prod[:].rearrange("p (o c) -> p o c", c=KC),
                                        axis=AX.X, op=OP.add)
                ht1.append(t)
            S["ht1"] = ht1

        def st_msg1(b, S):
            pbig, pg, pacc, prep = psets[b % 2]
            msgp = pacc.tile([D1, R], F32, tag="pacc")
            for jc, (jo, jn) in enumerate(CH):
                nc.tensor.matmul(msgp[:], S["ht1"][jc][:], S["alT"][jc][:],
                                 start=(jc == 0), stop=(jc == 3))
            hT1 = work.tile([D1, R], F32, tag="hT1")
            nc.vector.tensor_tensor(hT1[:], msgp[:], S["recip1"][:], op=OP.mult)
            nc.scalar.activation(hT1[:], hT1[:], AF.Identity, bias=b1t[:])
            S["hT1"] = hT1
            srp = prep.tile([1, R], F32, tag="prep")
            nc.tensor.matmul(srp[:], p1t[:], hT1[:])
            y1row = work.tile([1, R], F32, tag="y1row")
            nc.scalar.activation(y1row[:], srp[:], AF.Identity)
            S["y1row"] = y1row

        def row_to_col_bcast(b, yrow, kname):
            pbig, pg, pacc, prep = psets[b % 2]
            ycp = prep.tile([128, 4], F32, tag="prep")
            for mc, (mo, mn) in enumerate(CH):
                nc.tensor.matmul(ycp[:mn, mc:mc + 1], yrow[:, mo:mo + mn],
                                 ones_r[:, 0:1])
            yc = work.tile([128, 4], F32, tag=f"{kname}_yc")
            nc.vector.memset(yc[:, 3:4], 0.0)
            nc.scalar.activation(yc[:, 0:3], ycp[:, 0:3], AF.Identity)
            nc.scalar.activation(yc[:16, 3:4], ycp[:16, 3:4], AF.Identity)
            srep = prep.tile([128, R], F32, tag="prep")
            nc.tensor.matmul(srep[:], ones_r[:], yrow[:])
            return yc, srep

        def rank_keep(b, srep_t, ycol_t, thresh_row, thresh_col, kname):
            pbig, pg, pacc, prep = psets[b % 2]
            csp = prep.tile([1, R], F32, tag="prep")
            rank4 = work.tile([128, 4], F32, tag=f"{kname}_rk")
            nc.vector.memset(rank4[:, 3:4], 999.0)
            for ic, (io, inn) in enumerate(CH):
                cmp = work.tile([128, R], BF16, tag=f"cmp{ic}")
                nc.vector.tensor_scalar(cmp[:inn, :], srep_t[:inn, :],
                                        ycol_t[:inn, ic:ic + 1],
                                        0.0, op0=OP.is_gt, op1=OP.add,
                                        accum_out=rank4[:inn, ic:ic + 1])
                nc.tensor.matmul(csp[:], ones_bf[:inn, :1], cmp[:inn, :],
                                 start=(ic == 0), stop=(ic == 3))
            keep_row = work.tile([1, R], F32, tag=f"{kname}_row")
            nc.vector.tensor_scalar(keep_row[:], csp[:], thresh_row, None,
                                    op0=OP.is_gt)
            keep_rowb = work.tile([1, R], BF16, tag=f"{kname}_rowb")
            nc.vector.tensor_scalar(keep_rowb[:], csp[:], thresh_row, None,
                                    op0=OP.is_gt)
            keep_col = work.tile([128, 4], F32, tag=f"{kname}_col")
            nc.vector.tensor_scalar(keep_col[:], rank4[:], thresh_col, None,
                                    op0=OP.is_lt)
            return keep_row, keep_rowb, keep_col

        def readout(b, hk_t, krep_t, kdiv, zoff):
            mx = work.tile([D1, R], F32, tag="mx")
            nc.vector.scalar_tensor_tensor(mx[:], krep_t[:], BIG, hk_t[:],
                                           op0=OP.mult, op1=OP.add)
            red = work.tile([D1, 2], F32, tag="red")
            nc.vector.tensor_reduce(red[:, 0:1], mx[:], axis=AX.X, op=OP.max)
            nc.vector.tensor_reduce(red[:, 1:2], hk_t[:], axis=AX.X, op=OP.add)
            nc.vector.tensor_scalar(ztile[zoff:zoff + D1, b:b + 1], red[:, 0:1],
                                    -BIG, None, op0=OP.add)
            nc.vector.tensor_scalar(ztile[zoff + D1:zoff + 2 * D1, b:b + 1],
                                    red[:, 1:2], 1.0 / kdiv, None, op0=OP.mult)

        def st_pool1(b, S):
            pbig, pg, pacc, prep = psets[b % 2]
            y1c, srep1 = row_to_col_bcast(b, S["y1row"], "k1")
            keep_row, keep_rowb, keep_col = rank_keep(
                b, srep1, y1c, float(R - 1 - K1) + 0.5, K1 - 0.5, "k1")
            S["keep_row"], S["keep_rowb"], S["keep_col"] = keep_row, keep_rowb, keep_col
            s_rowv = work.tile([1, R], F32, tag="s_rowv")
            nc.scalar.activation(s_rowv[:], S["y1row"][:], AF.Sigmoid)
            sk_row = work.tile([1, R], F32, tag="sk_row")
            nc.vector.tensor_tensor(sk_row[:], s_rowv[:], keep_row[:], op=OP.mult)
            skrepD = prep.tile([D1, R], F32, tag="prep")
            nc.tensor.matmul(skrepD[:], ones_r[:, :D1], sk_row[:])
            krep = pacc.tile([D1, R], F32, tag="pacc")
            nc.tensor.matmul(krep[:], ones_rb[:], keep_rowb[:])
            hk = work.tile([D1, R], F32, tag="hk")
            nc.vector.tensor_tensor(hk[:], S["hT1"][:], skrepD[:], op=OP.mult)
            S["hk"] = hk
            readout(b, hk, krep, K1, 0)

        def st_struct(b, S):
            pbig, pg, pacc, prep = psets[b % 2]
            keep_col = S["keep_col"]
            indalk = []
            for jc, (jo, jn) in enumerate(CH):
                t = work.tile([jn, R], BF16, tag=f"indalk{jc}")
                nc.gpsimd.tensor_scalar(t[:], S["at"][jc][:], 0.0,
                                        keep_col[:jn, jc:jc + 1],
                                        op0=OP.is_gt, op1=OP.mult)
                indalk.append(t)
            cnt2p = pacc.tile([D2, R], F32, tag="pacc")
            for uc, (uo, un) in enumerate(CH):
                qp = pbig.tile([un, R], F32, tag="pT")
                for jc, (jo, jn) in enumerate(CH):
                    nc.tensor.matmul(qp[:], indalk[jc][:, uo:uo + un], S["ind"][jc][:],
                                     start=(jc == 0), stop=(jc == 3))
                # no diag fix needed for the indicator: P[u,u] >= keep_u
                # (the self-path al[u,u]=1 counts itself), so (P>0)*keep
                # already equals the diag-corrected m2.
                t = work.tile([un, R], BF16, tag="ind2")
                nc.vector.tensor_scalar(t[:], qp[:], 0.0,
                                        keep_col[:un, uc:uc + 1],
                                        op0=OP.is_gt, op1=OP.mult)
                nc.tensor.matmul(cnt2p[:], ones_bf[:un, :D2], t[:],
                                 start=(uc == 0), stop=(uc == 3))
            cnt2s = work.tile([D2, R], F32, tag="cnt2s")
            nc.vector.tensor_scalar(cnt2s[:], cnt2p[:], 1.0, None, op0=OP.max)
            recip2 = work.tile([D2, R], F32, tag="recip2")
            nc.vector.reciprocal(recip2[:], cnt2s[:])
            S["recip2"] = recip2

        def st_conv2(b, S):
            pbig, pg, pacc, prep = psets[b % 2]
            ht2 = []
            for mc, (mo, mn) in enumerate(CH):
                gp = pg.tile([mn, D2 * KC], F32, tag="pG")
                nc.tensor.matmul(gp[:], S["hk"][:, mo:mo + mn], bc2f[:])
                prod = work.tile([mn, D2 * KC], F32, tag="prod")
                abc = a2t[mc][:].unsqueeze(1).broadcast_to((mn, D2, KC))
                nc.vector.tensor_tensor(prod[:].rearrange("p (o c) -> p o c", c=KC),
                                        gp[:].rearrange("p (o c) -> p o c", c=KC),
                                        abc, op=OP.mult)
                t = work.tile([mn, D2], F32, tag=f"ht2_{mc}")
                nc.vector.tensor_reduce(t[:], prod[:].rearrange("p (o c) -> p o c", c=KC),
                                        axis=AX.X, op=OP.add)
                ht2.append(t)
            S["ht2"] = ht2

        def st_msg2(b, S):
            pbig, pg, pacc, prep = psets[b % 2]
            keep_col = S["keep_col"]
            ht2, alT, at = S["ht2"], S["alT"], S["at"]
            tks = []
            for jc, (jo, jn) in enumerate(CH):
                tp = prep.tile([128, D2], F32, tag="prep")
                for uc, (uo, un) in enumerate(CH):
                    nc.tensor.matmul(tp[:jn, :], alT[uc][:, jo:jo + jn],
                                     ht2[uc][:], start=(uc == 0), stop=(uc == 3))
                tk = work.tile([jn, D2], F32, tag=f"tk{jc}")
                nc.scalar.activation(tk[:], tp[:jn, :], AF.Identity,
                                     scale=keep_col[:jn, jc:jc + 1])
                tks.append(tk)
            msg2p = pacc.tile([D2, R], F32, tag="pacc")
            for jc, (jo, jn) in enumerate(CH):
                nc.tensor.matmul(msg2p[:], tks[jc][:], alT[jc][:],
                                 start=(jc == 0), stop=(jc == 3))
            ht2T = work.tile([D2, R], F32, tag="ht2T")
            for mc, (mo, mn) in enumerate(CH):
                tp = prep.tile([D2, 128], F32, tag="prep")
                nc.tensor.matmul(tp[:, :mn], ht2[mc][:], I128[:mn, :mn])
                nc.scalar.activation(ht2T[:, mo:mo + mn], tp[:, :mn], AF.Identity)
            Ets = []
            for jc, (jo, jn) in enumerate(CH):
                Et = work.tile([jn, R], F32, tag=f"Etmp{jc}")
                nc.gpsimd.tensor_tensor(Et[:], at[jc][:], alT[jc][:], op=OP.mult)
                Ets.append(Et)
            dcp = prep.tile([128, 4], F32, tag="prep")
            for mc, (mo, mn) in enumerate(CH):
                for jc, (jo, jn) in enumerate(CH):
                    nc.tensor.matmul(dcp[:mn, mc:mc + 1], Ets[jc][:, mo:mo + mn],
                                     keep_col[:jn, jc:jc + 1],
                                     start=(jc == 0), stop=(jc == 3))
            d_col = work.tile([128, 4], F32, tag="d_col")
            nc.vector.memset(d_col[:, 3:4], 0.0)
            nc.scalar.activation(d_col[:, 0:3], dcp[:, 0:3], AF.Identity)
            nc.scalar.activation(d_col[:16, 3:4], dcp[:16, 3:4], AF.Identity)
            drp = prep.tile([1, R], F32, tag="prep")
            for mc, (mo, mn) in enumerate(CH):
                nc.tensor.matmul(drp[:, mo:mo + mn], d_col[:mn, mc:mc + 1],
                                 I128[:mn, :mn])
            kd_row = work.tile([1, R], F32, tag="kd_row")
            nc.vector.tensor_tensor(kd_row[:], S["keep_row"][:], drp[:], op=OP.subtract)
            kdrep = prep.tile([D2, R], F32, tag="prep")
            nc.tensor.matmul(kdrep[:], ones_r[:, :D2], kd_row[:])
            hT2 = work.tile([D2, R], F32, tag="hT2")
            nc.vector.tensor_tensor(hT2[:], kdrep[:], ht2T[:], op=OP.mult)
            nc.vector.tensor_tensor(hT2[:], hT2[:], msg2p[:], op=OP.add)
            nc.vector.tensor_tensor(hT2[:], hT2[:], S["recip2"][:], op=OP.mult)
            nc.scalar.activation(hT2[:], hT2[:], AF.Identity, bias=b2t[:])
            S["hT2"] = hT2
            srp2 = prep.tile([1, R], F32, tag="prep")
            nc.tensor.matmul(srp2[:], p2t[:], hT2[:])
            y2row = work.tile([1, R], F32, tag="y2row")
            nc.vector.tensor_tensor(y2row[:], srp2[:], S["keep_row"][:], op=OP.mult)
            mneg = work.tile([1, R], F32, tag="mneg")
            nc.vector.tensor_scalar(mneg[:], S["keep_row"][:], 1.0, CBIG,
                                    op0=OP.subtract, op1=OP.mult)
            nc.vector.tensor_tensor(y2row[:], y2row[:], mneg[:], op=OP.add)
            S["y2row"] = y2row

        def st_pool2(b, S):
            pbig, pg, pacc, prep = psets[b % 2]
            y2row = S["y2row"]
            y2c, srep2 = row_to_col_bcast(b, y2row, "k2")
            keep2_row, keep2_rowb, _k2c = rank_keep(
                b, srep2, y2c, float(R - 1 - K2) + 0.5, K2 - 0.5, "k2")
            s2_rowv = work.tile([1, R], F32, tag="s2_rowv")
            nc.scalar.activation(s2_rowv[:], y2row[:], AF.Sigmoid)
            sk2_row = work.tile([1, R], F32, tag="sk2_row")
            nc.vector.tensor_tensor(sk2_row[:], s2_rowv[:], keep2_row[:], op=OP.mult)
            skrep2D = prep.tile([D2, R], F32, tag="prep")
            nc.tensor.matmul(skrep2D[:], ones_r[:, :D2], sk2_row[:])
            krep2 = pacc.tile([D2, R], F32, tag="pacc")
            nc.tensor.matmul(krep2[:], ones_rb[:], keep2_rowb[:])
            hk2 = work.tile([D2, R], F32, tag="hk2")
            nc.vector.tensor_tensor(hk2[:], S["hT2"][:], skrep2D[:], op=OP.mult)
            readout(b, hk2, krep2, K2, 2 * D1)

        stages = [st_load, st_cnt_conv1, st_msg1, st_pool1, st_struct,
                  st_conv2, st_msg2, st_pool2]
        NST = len(stages)
        DELTA = 8   # stage skew between consecutive graphs
        states = {}
        for s in range(DELTA * (BL - 1) + NST):
            for b in range(BL):
                k = s - DELTA * b
                if 0 <= k < NST:
                    if k == 0:
                        states[b] = {}
                    stages[k](b, states[b])
                    if k == NST - 1:
                        del states[b]

        # ---------------- AllGather + head (redundant on every core) --------
        zloc = dram.tile([128, BL], F32)
        zag = dram.tile([128 * n_cores, BL], F32)
        nc.gpsimd.dma_start(zloc[:], ztile[:])
        nc.gpsimd.collective_compute(
            "AllGather",
            mybir.AluOpType.bypass,
            replica_groups=[list(range(n_cores))],
            ins=[zloc[:].opt()],
            outs=[zag[:].opt()],
        )
        ZT = cons.tile([128, B], F32, tag="ZT")
        nc.sync.dma_start(ZT[:].rearrange("p (c b) -> p c b", b=BL),
                          zag[:].rearrange("(c p) b -> p c b", p=128))

        def bn(y, n, gain, beta):
            mu = cons.tile([n, 1], F32, tag="bn_mu")
            nc.vector.tensor_reduce(mu[:], y[:], axis=AX.X, op=OP.add)
            nc.vector.tensor_scalar(mu[:], mu[:], 1.0 / B, None, op0=OP.mult)
            cen = cons.tile([n, B], F32, tag="bn_cen")
            nc.vector.tensor_scalar(cen[:], y[:], mu[:], None, op0=OP.subtract)
            sq = cons.tile([n, B], F32, tag="bn_sq")
            nc.vector.tensor_tensor(sq[:], cen[:], cen[:], op=OP.mult)
            var = cons.tile([n, 1], F32, tag="bn_var")
            nc.vector.tensor_reduce(var[:], sq[:], axis=AX.X, op=OP.add)
            rstd = cons.tile([n, 1], F32, tag="bn_rstd")
            nc.scalar.activation(rstd[:], var[:], AF.Sqrt, bias=eps128[:n, :],
                                 scale=1.0 / B)
            nc.vector.reciprocal(rstd[:], rstd[:])
            gn = cons.tile([n, 1], F32, tag="bn_gn")
            nc.vector.tensor_tensor(gn[:], rstd[:], gain, op=OP.mult)
            nc.vector.tensor_scalar(y[:], cen[:], gn[:], beta, op0=OP.mult, op1=OP.add)

        y1p = pg.tile([D2, B], F32, tag="pG")
        nc.tensor.matmul(y1p[:], fc1wt[:], ZT[:])
        y1 = cons.tile([D2, B], F32, tag="y1")
        nc.scalar.activation(y1[:], y1p[:], AF.Relu, bias=fc1bt[:])
        bn(y1, D2, g1t[:], be1t[:])

        y3p = pacc.tile([2, B], F32, tag="pacc")
        for mc in range(4):
            y2p = pg.tile([128, B], F32, tag="pG")
            nc.tensor.matmul(y2p[:], fc2wt[:, 128 * mc:128 * (mc + 1)], y1[:])
            y2 = cons.tile([128, B], F32, tag="y2")
            nc.scalar.activation(y2[:], y2p[:], AF.Relu, bias=fc2b4[:, mc:mc + 1])
            bn(y2, 128, g24[:, mc:mc + 1], be24[:, mc:mc + 1])
            nc.tensor.matmul(y3p[:], fc3wt[:, 2 * mc:2 * (mc + 1)], y2[:],
                             start=(mc == 0), stop=(mc == 3))
        y3 = cons.tile([2, B], F32, tag="y3")
        nc.scalar.activation(y3[:], y3p[:], AF.Identity, bias=fc3bt[:])
        nc.sync.dma_start(outd[:, :].rearrange("b o -> o b"), y3[:])

    return nc


def make_in_maps(inputs, n_cores=NCORES):
    f32 = np.float32
    x = np.ascontiguousarray(inputs["x"], dtype=f32)
    adj = np.ascontiguousarray(inputs["adj_w"], dtype=f32)
    shared = {
        "w1a": np.ascontiguousarray(inputs["W1a"], f32),
        "bc1": np.ascontiguousarray(
            inputs["W1b"].reshape(KC, R, D1).transpose(1, 2, 0).reshape(R, D1 * KC), f32),
        "b1d": np.ascontiguousarray(inputs["b1"], f32),
        "p1d": np.ascontiguousarray(inputs["p1"] / np.linalg.norm(inputs["p1"]), f32),
        "w2a": np.ascontiguousarray(inputs["W2a"], f32),
        "bc2": np.ascontiguousarray(
            inputs["W2b"].reshape(KC, D1, D2).transpose(1, 2, 0).reshape(D1, D2 * KC), f32),
        "b2d": np.ascontiguousarray(inputs["b2"], f32),
        "p2d": np.ascontiguousarray(inputs["p2"] / np.linalg.norm(inputs["p2"]), f32),
        "fc1wd": np.ascontiguousarray(inputs["fc1_w"], f32),
        "fc1bd": np.ascontiguousarray(inputs["fc1_b"], f32),
        "g1d": np.ascontiguousarray(inputs["g1"], f32),
        "be1d": np.ascontiguousarray(inputs["be1"], f32),
        "fc2wd": np.ascontiguousarray(inputs["fc2_w"], f32),
        "fc2bd": np.ascontiguousarray(inputs["fc2_b"], f32),
        "g2d": np.ascontiguousarray(inputs["g2"], f32),
        "be2d": np.ascontiguousarray(inputs["be2"], f32),
        "fc3wd": np.ascontiguousarray(inputs["fc3_w"], f32),
        "fc3bd": np.ascontiguousarray(inputs["fc3_b"], f32),
    }
    maps = []
    eye = np.eye(R, dtype=f32)
    for c in range(n_cores):
        m = dict(shared)
        m["xl"] = np.ascontiguousarray(x[c * BL:(c + 1) * BL])
        # adjacency shipped with self-loops already added (A + I)
        alc = adj[c * BL:(c + 1) * BL] + eye
        m["al"] = np.ascontiguousarray(alc)
        m["altd"] = np.ascontiguousarray(alc.transpose(0, 2, 1))
        maps.append(m)
    return maps


_CACHED = {}


def _legalize_waits(nc, dummy_sem, cap_ldw=2):
    """Walrus on this image encodes at most ONE sync wait per instruction
    (EventSemaphore: 2, except on the SP queue where only wait+update
    single-wait ESes assemble).  Bass's own legalizer targets a 2-wait
    budget, so spill the extras onto carrier instructions inserted just
    before the over-subscribed instruction on the same engine queue:
      - PE: onto the preceding update-free Ldweights (or synthesize a
        dummy one re-loading the matmul's own stationary operand —
        harmless, every matmul here self-loads).
      - other queues: EventSemaphore carriers with a scratch-semaphore
        increment (CoreSim requires an update; wait-only ESes also
        mis-assemble on the SP sequencer).
    Moving a wait to the immediately preceding same-queue slot only
    stalls the queue earlier, and the carriers update nothing another
    engine could wait on, so no deadlock can be introduced."""
    from concourse import mybir
    import bass_rust as _br

    ctr = 0
    for fn in nc.m.functions:
        for blk in fn.blocks:
            newinsts = []
            changed = False
            for inst in blk.instructions:
                si = inst.sync_info
                tname = type(inst).__name__
                engine = inst.engine
                is_pe = engine == mybir.EngineType.PE
                is_sp = engine == mybir.EngineType.SP
                cap_es = 1 if is_sp else 2
                cap = cap_es if tname == 'InstEventSemaphore' else 1
                if si is None or len(si.on_wait) <= cap:
                    newinsts.append(inst)
                    continue
                waits = list(si.on_wait)
                extras, keep = waits[:-cap], waits[-cap:]

                def dummy_upd():
                    return [_br.SyncUpdate(
                        sync_type='semaphore', id=dummy_sem.num,
                        ant_name=dummy_sem.name,
                        update_mode='sem-inc', update_value=1)]

                if is_pe:
                    prev = newinsts[-1] if newinsts else None
                    if (len(extras) == 1 and prev is not None
                            and type(prev).__name__ == 'InstLdweights'
                            and prev.engine == engine
                            and prev.sync_info is not None
                            and not prev.sync_info.on_update
                            and not prev.sync_info.on_wait):
                        prev.sync_info = mybir.SyncInfo(
                            on_wait=extras, on_update=[])
                    else:
                        # one 1-wait dummy Ldweights per extra (LW struct
                        # also encodes a single wait)
                        for w in extras:
                            ldw = mybir.InstLdweights(
                                name=f"lgl_ldw_{ctr}", ins=[inst.ins[1]],
                                outs=[])
                            ctr += 1
                            ldw.engine = engine
                            ldw.sync_info = mybir.SyncInfo(
                                on_wait=[w], on_update=dummy_upd())
                            newinsts.append(ldw)
                else:
                    while extras:
                        ch, extras = extras[:cap_es], extras[cap_es:]
                        es = mybir.InstEventSemaphore(
                            name=f"lgl_es_{ctr}", ins=[], outs=[])
                        ctr += 1
                        es.engine = engine
                        es.sync_info = mybir.SyncInfo(
                            on_wait=ch, on_update=dummy_upd())
                        newinsts.append(es)
                si.on_wait = keep
                newinsts.append(inst)
                changed = True
            if changed:
                blk.instructions = newinsts
    return ctr


def _build_legalized():
    nc = build_nc(NCORES)
    nc.finalize()
    _legalize_waits(nc, nc._lgl_dummy_sem)
    return nc


def _run_sim(in_maps):
    # Fallback executor: 8-core CoreSim of the same BIR.
    from concourse import bass_interp

    nc = _build_legalized()
    sim = bass_interp.MultiCoreSim(nc, NCORES, num_workers=1)
    for i in range(NCORES):
        for k, v in in_maps[i].items():
            sim.cores[i].tensor(k)[:] = v
    sim.simulate()
    return np.array(sim.cores[0].tensor("out"), dtype=np.float32)


def kernel(**inputs):
    in_maps = make_in_maps(inputs, NCORES)
    try:
        from concourse.bass_utils import run_bass_kernel_spmd

        if "nc" not in _CACHED:
            _CACHED["nc"] = _build_legalized()
        res = run_bass_kernel_spmd(_CACHED["nc"], in_maps, list(range(NCORES)))
        return np.asarray(res.results[0]["out"], dtype=np.float32)
    except Exception:
        return _run_sim(in_maps)
